# revision 27
# baseline (speedup 1.0000x reference)
"""3-layer GAT forward for nn_GAT_21045339750566 on 8 TRN2 NeuronCores.

Redesign vs baseline: gather bf16 *pre-transformed* per-head features
(table rows [h@W+b | el]) so edge aggregation is ONE 260-wide bf16 matmul
per 128-edge chunk; one-hot edge/dst matrices (pure graph structure) are
host-precomputed and DMA'd; biases and attention vectors are folded on
host; next-layer table build is fused into each window's finalize.

Hardcoded problem shape: N=50000 nodes, E=800000 edges, F=256, H=4 heads,
D=64, C=40 classes, 8 cores.
"""
import os
import sys
import numpy as np

sys.path.insert(0, '/opt/trn_rl_repo')

from concourse import mybir

MAX_WAITS = 1


def legalize_waits(nc, max_waits=MAX_WAITS):
    """Walrus on this stack rejects instructions carrying more than MAX_WAITS
    sem waits. Hoist excess waits onto InstNoOp instructions inserted just
    before the offending instruction (same engine, program order preserved)."""
    n_fixed = 0
    for fn in nc.m.functions:
        for blk in fn.blocks:
            il = blk.instructions
            i = 0
            while i < len(il):
                inst = il[i]
                si = inst.sync_info
                if si is not None and len(si.on_wait) > max_waits:
                    waits = list(si.on_wait)
                    keep = waits[-max_waits:]
                    extra = waits[:-max_waits]
                    inst.sync_info = mybir.SyncInfo(
                        on_wait=keep, on_update=list(si.on_update)
                    )
                    nops = []
                    for j in range(0, len(extra), max_waits):
                        nop = mybir.InstNoOp(
                            name=nc.get_next_instruction_name(),
                            engine=inst.engine,
                            bass_nofuse=True,
                            sync_info=mybir.SyncInfo(
                                on_wait=extra[j : j + max_waits], on_update=[]
                            ),
                        )
                        try:
                            nc.register_instruction(nop)
                        except Exception:
                            pass
                        nops.append(nop)
                    for k, nop in enumerate(nops):
                        il.insert(i + k, nop)
                    i += len(nops)
                    n_fixed += 1
                i += 1
    return n_fixed


import concourse.bass as bass
import concourse.tile as tile
from concourse import library_config
from concourse.library_overlay import lower_extended_insts
import ml_dtypes

BF16_NP = ml_dtypes.bfloat16
F32 = mybir.dt.float32
BF16 = mybir.dt.bfloat16
I16 = mybir.dt.int16
AF = mybir.ActivationFunctionType
OP = mybir.AluOpType
AX = mybir.AxisListType

MAXG = 2048
WIN = 128
NEG_SLOPE = 0.2
SPLIT = 32768
AGCH = 7           # windows per AllGather chunk (nwin = 49 = 7*7)
CHR = AGCH * WIN   # own rows per chunk (896)


class Meta:
    pass


def build_meta(src, dst, N, n_cores):
    """Per-core edge metadata. Per-core edge order: windows ascending; within
    a window group A (src_pad < SPLIT) then group B, each padded to a
    multiple of 128 (uniform max over cores). Pad gather idxs are -1 (the Q7
    truncates trailing negatives per core); pad slots have all-zero one-hot
    columns so they contribute nothing."""
    shard = N // n_cores
    nwin = (shard + WIN - 1) // WIN
    m = Meta()
    shard_pad = nwin * WIN
    m.N, m.n_cores, m.shard, m.nwin = N, n_cores, shard, nwin
    m.shard_pad = shard_pad
    m.N_pad = n_cores * shard_pad
    # Table rows are chunk-major so each AllGather chunk writes a contiguous
    # block: node (core g, local n) -> row (n//CHR)*(n_cores*CHR) + g*CHR + n%CHR
    g, n = src // shard, src % shard
    src = (n // CHR) * (n_cores * CHR) + g * CHR + (n % CHR)

    pcw = []   # [core][win] -> (srcA, srcB-SPLIT, dlocA, dlocB)
    for c in range(n_cores):
        sel = (dst // shard) == c
        s_c, d_c = src[sel], dst[sel]
        dloc = (d_c - c * shard).astype(np.int64)
        order = np.argsort(dloc, kind='stable')
        s_c, dloc = s_c[order], dloc[order]
        wins = []
        for w in range(nwin):
            lo, hi = np.searchsorted(dloc, [w * WIN, (w + 1) * WIN])
            sw, dw = s_c[lo:hi], dloc[lo:hi] - w * WIN
            a = sw < SPLIT
            wins.append((sw[a], sw[~a] - SPLIT, dw[a], dw[~a]))
        pcw.append(wins)

    up = lambda n: max(-(-n // 128) * 128, 0)
    nA = [max(128, max(up(len(pcw[c][w][0])) for c in range(n_cores)))
          for w in range(nwin)]
    nB = [max(up(len(pcw[c][w][1])) for c in range(n_cores)) for w in range(nwin)]

    m.win_desc = []
    icol = 0
    for w in range(nwin):
        m.win_desc.append(dict(nA=nA[w], nB=nB[w], offA=icol,
                               offB=icol + nA[w] // 16, nE=nA[w] + nB[w],
                               nch=(nA[w] + nB[w]) // 128))
        icol += (nA[w] + nB[w]) // 16
    m.tot_icols = icol
    m.maxE = max(d['nE'] for d in m.win_desc)
    m.maxch = m.maxE // 128

    def wrap16(idx):
        return np.tile(idx.reshape(-1, 16).T, (8, 1)).astype(np.int16)

    # Pad gather idxs with 0 (a valid row: every slot gets written, padded
    # slots are killed by their all-zero one-hot columns). Trailing -1
    # padding (Q7-truncated, saves desc-gen) crashes this HW stack.
    m.idx16, m.mt, m.mprc = [], [], []
    for c in range(n_cores):
        i16 = np.full((128, m.tot_icols), -1, np.int16)
        mt = np.zeros((nwin * 128, m.maxE), BF16_NP)
        mprc = np.zeros((nwin * 128, m.maxE), BF16_NP)
        for w in range(nwin):
            pad = 0
            sA, sB, dA, dB = pcw[c][w]
            d = m.win_desc[w]
            a = np.full(d['nA'], pad, np.int64); a[:len(sA)] = sA
            b = np.full(d['nB'], pad, np.int64); b[:len(sB)] = sB
            i16[:, d['offA']:d['offA'] + d['nA'] // 16] = wrap16(a)
            if d['nB']:
                i16[:, d['offB']:d['offB'] + d['nB'] // 16] = wrap16(b)
            dl = np.full(d['nE'], -1, np.int64)
            dl[:len(dA)] = dA
            dl[d['nA']:d['nA'] + len(dB)] = dB
            slots = np.nonzero(dl >= 0)[0]
            dv = dl[slots]
            # mt[j, s] = (dstloc[s] == j)
            mt[w * 128 + dv, slots] = 1
            # mprc[p, c*128 + j] = (slot c*128+p has dstloc == j)
            mprc[w * 128 + (slots % 128), (slots // 128) * 128 + dv] = 1
        m.idx16.append(i16)
        m.mt.append(mt)
        m.mprc.append(mprc)
    return m


def blockdiag(al, heads, dim):
    out = np.zeros((heads * dim, heads), np.float64)
    for h in range(heads):
        out[h * dim:(h + 1) * dim, h] = al[h]
    return out


def chunk_rows(a):
    """[K*128, W] -> [128, K*W] (row-chunk k at cols k*W)."""
    K = a.shape[0] // 128
    W = a.shape[1]
    out = np.zeros((128, K * W), a.dtype)
    for k in range(K):
        out[:, k * W:(k + 1) * W] = a[k * 128:(k + 1) * 128]
    return out


def fold_weights(W, al, ar, b, heads, dim):
    """-> (Wcat [Fin, Fout+2H] f64, brow [Fout+2H] f64)."""
    W = np.asarray(W, np.float64)
    b = np.asarray(b, np.float64).reshape(-1)
    bdl = blockdiag(np.asarray(al, np.float64), heads, dim)
    bdr = blockdiag(np.asarray(ar, np.float64), heads, dim)
    Vl, Vr = W @ bdl, W @ bdr
    bl = b @ bdl
    Wcat = np.concatenate([W, Vl, Vr], 1)
    brow = np.concatenate([b, bl, -bl])
    return Wcat, brow


def build_kernel(nc, meta, F, H, Dh, C):
    nwin, shard_pad, N_pad = meta.nwin, meta.shard_pad, meta.N_pad
    maxE, maxch = meta.maxE, meta.maxch
    ROW1 = 384   # bf16 slots per L1/L2 table row: [h(256) | el f32 (4->8 slots) | pad]
    ROW3 = 128   # L3: [h(40) | el f32 (1->2 slots) | pad]
    W12 = F + 2 * H       # 264 table-build matmul width
    W3 = C + 2            # 42
    AGG12 = F + H         # 260 agg rhs width
    AGG3 = C + 1          # 41

    io = {}
    def inp(name, shape, dtype=F32):
        io[name] = nc.dram_tensor(name, shape, dtype, kind="ExternalInput")
        return io[name]

    XT = inp("xT_own", [128, 2 * shard_pad], BF16)
    W1C = inp("w1cat", [128, 2 * W12], BF16)
    W2C = inp("w2cat", [128, 2 * W12], BF16)
    W3C = inp("w3cat", [128, 2 * W3], BF16)
    B1R = inp("b1row", [1, W12], BF16)
    B2R = inp("b2row", [1, W12], BF16)
    B3R = inp("b3row", [1, W3], BF16)
    IDX = inp("idx16", [128, meta.tot_icols], I16)
    MT = inp("mt", [nwin * 128, maxE], BF16)
    MPRC = inp("mprc", [nwin * 128, maxE], BF16)
    ONES1 = inp("ones1", [1, 128], BF16)
    IDENT = inp("ident", [128, 128])
    OUT = nc.dram_tensor("out", [shard_pad, C], F32, kind="ExternalOutput")

    nck = nwin // AGCH
    t1_own = [nc.dram_tensor(f"t1_own{k}", [CHR, ROW1], BF16) for k in range(nck)]
    t1_full = nc.dram_tensor("t1_full", [N_pad, ROW1], BF16, addr_space="Shared")
    t2_own = [nc.dram_tensor(f"t2_own{k}", [CHR, ROW1], BF16) for k in range(nck)]
    t2_full = nc.dram_tensor("t2_full", [N_pad, ROW1], BF16, addr_space="Shared")
    t3_own = [nc.dram_tensor(f"t3_own{k}", [CHR, ROW3], BF16) for k in range(nck)]
    t3_full = nc.dram_tensor("t3_full", [N_pad, ROW3], BF16, addr_space="Shared")
    er1_tab = nc.dram_tensor("er1_tab", [shard_pad, H], BF16)
    er2_tab = nc.dram_tensor("er2_tab", [shard_pad, H], BF16)
    er3_tab = nc.dram_tensor("er3_tab", [shard_pad, 1], BF16)

    _dbg = os.environ.get("GAT_DEBUG") == "1"
    if _dbg:
        DBG = nc.dram_tensor("dbg", [128, 4096], F32, kind="ExternalOutput")
        io['_dbg'] = True
        _dbg_state = {'n': 0}

    reg_cache = {}
    def reg(v):
        if v not in reg_cache:
            reg_cache[v] = nc.gpsimd.to_reg(v)
        return reg_cache[v]

    with tile.TileContext(nc) as tc:
        with tc.tile_pool(name="cst", bufs=1) as cst:
            nc.gpsimd.load_library(library_config.mlp)

            def load_const(name, shape, dtype=F32):
                tl = cst.tile(shape, dtype, tag=name)
                nc.sync.dma_start(out=tl[:], in_=io[name][:])
                return tl

            ident = load_const("ident", [128, 128])
            ones1 = load_const("ones1", [1, 128], BF16)
            w1c = load_const("w1cat", [128, 2 * W12], BF16)
            w2c = load_const("w2cat", [128, 2 * W12], BF16)
            w3c = load_const("w3cat", [128, 2 * W3], BF16)
            b1r = load_const("b1row", [1, W12], BF16)
            b2r = load_const("b2row", [1, W12], BF16)
            b3r = load_const("b3row", [1, W3], BF16)
            idx_sb = load_const("idx16", [128, meta.tot_icols], I16)

            def dbg_dump(ap, cols, pool):
                """Copy [128, cols] f32-castable AP into DBG columns."""
                if not _dbg:
                    return
                n = _dbg_state['n']
                if n + cols > 4096:
                    return
                t = pool.tile([128, cols], F32, tag=f"dbg{n}")
                nc.vector.tensor_copy(out=t[:], in_=ap)
                nc.sync.dma_start(out=DBG[:, n:n + cols], in_=t[:])
                _dbg_state['n'] = n + cols

            # table-build: psum[128, W] = xnT.T @ Wcat + 1 x brow
            def build_rows(xnT, wc, br, Wc, ppool):
                pb = ppool.tile([128, 512], F32, tag="pbuild")
                for k in range(2):
                    nc.tensor.matmul(out=pb[:, :Wc], lhsT=xnT[:, k * 128:(k + 1) * 128],
                                     rhs=wc[:, k * Wc:(k + 1) * Wc],
                                     start=(k == 0), stop=False)
                nc.tensor.matmul(out=pb[:, :Wc], lhsT=ones1[:], rhs=br[:],
                                 start=False, stop=True)
                return pb

            # pack psum rows -> bf16 table row tile + er tile, DMA out
            def pack_rows(pb, feats, heads, row_w, trow, erow, t_dst, er_dst, w):
                nc.vector.tensor_copy(out=trow[:, 0:feats], in_=pb[:, 0:feats])
                trow_f32 = trow[:].bitcast(F32)
                elc = feats // 2
                nc.vector.tensor_copy(out=trow_f32[:, elc:elc + heads],
                                      in_=pb[:, feats:feats + heads])
                nc.vector.memset(trow[:, feats + 2 * heads:row_w], 0.0)
                nc.vector.tensor_copy(out=erow[:], in_=pb[:, feats + heads:feats + 2 * heads])
                r0 = (w % AGCH) * 128
                nc.sync.dma_start(out=t_dst[w // AGCH][r0:r0 + 128, :], in_=trow[:])
                nc.sync.dma_start(out=er_dst[w * 128:(w + 1) * 128, :], in_=erow[:])

            _stage = int(os.environ.get("GAT_STAGE", "4"))

            def make_ag(t_own, t_full_t):
                def ag(k):
                    f0 = k * meta.n_cores * CHR
                    nc.gpsimd.collective_compute(
                        "AllGather", OP.bypass,
                        replica_groups=[list(range(meta.n_cores))],
                        ins=[t_own[k][:]],
                        outs=[t_full_t[f0:f0 + meta.n_cores * CHR, :]])
                return ag

            ag1 = make_ag(t1_own, t1_full) if _stage >= 1 else None
            ag2 = make_ag(t2_own, t2_full) if _stage >= 3 else None
            ag3 = make_ag(t3_own, t3_full) if _stage >= 4 else None

            # ---------------- Phase T1: own-shard table build ----------------
            with tc.tile_pool(name="bp", bufs=3) as bp, \
                 tc.tile_pool(name="bpp", bufs=2, space="PSUM") as bpp:
                for t in range(nwin):
                    xtt = bp.tile([128, 256], BF16, tag="xtt")
                    xt3 = XT[:].rearrange("p (a n) -> p a n", a=2)
                    nc.sync.dma_start(out=xtt[:].rearrange("p (a n) -> p a n", a=2),
                                      in_=xt3[:, :, t * 128:(t + 1) * 128])
                    pb = build_rows(xtt, w1c, b1r, W12, bpp)
                    trow = bp.tile([128, ROW1], BF16, tag="trow")
                    erow = bp.tile([128, H], BF16, tag="erow")
                    pack_rows(pb, F, H, ROW1, trow, erow, t1_own, er1_tab, t)
                    if ag1 is not None and (t + 1) % AGCH == 0:
                        ag1((t + 1) // AGCH - 1)

            # ---------------- edge phase ----------------
            _ep = os.environ.get("GAT_EP", "full")  # gather|score|scaled|agg|full
            _ep_lvl = ["gather", "score", "scaled", "agg", "full"].index(_ep)

            def edge_phase(table, row_w, feats, heads, er_tab, fin, ag_fn=None):
                aggw = feats + heads
                with tc.tile_pool(name="gp", bufs=4) as gp, \
                     tc.tile_pool(name="wp", bufs=2) as wp, \
                     tc.tile_pool(name="mtp", bufs=2) as mtp, \
                     tc.tile_pool(name="p1", bufs=2, space="PSUM") as p1, \
                     tc.tile_pool(name="p2", bufs=2, space="PSUM") as p2:
                    for w in range(nwin):
                        d = meta.win_desc[w]
                        nch, nE = d['nch'], d['nE']
                        xg = gp.tile([128, maxch * row_w], BF16, tag="xg")
                        xg3 = xg[:].rearrange("p (c r) -> p c r", r=row_w)
                        for (cnt, coff, ch0, base) in (
                                (d['nA'], d['offA'], 0, 0),
                                (d['nB'], d['offB'], d['nA'] // 128, SPLIT)):
                            done = 0
                            while done < cnt:
                                step = min(MAXG, cnt - done)
                                src_ap = (table[0:SPLIT, :] if base == 0
                                          else table[SPLIT:N_pad, :])
                                nc.gpsimd.dma_gather(
                                    out_ap=xg3[:, ch0 + done // 128:
                                               ch0 + (done + step) // 128, :],
                                    in_ap=src_ap,
                                    idxs_ap=idx_sb[:, coff + done // 16:
                                                   coff + (done + step) // 16],
                                    num_idxs=step, num_idxs_reg=reg(step),
                                    elem_size=row_w, single_packet=False)
                                done += step
                        if _ep_lvl < 1:
                            continue
                        mt = mtp.tile([128, maxE], BF16, tag="mt")
                        nc.sync.dma_start(out=mt[:, :nE],
                                          in_=MT[w * 128:(w + 1) * 128, 0:nE])
                        mprc = mtp.tile([128, maxE], BF16, tag="mprc")
                        nc.sync.dma_start(out=mprc[:, :nE],
                                          in_=MPRC[w * 128:(w + 1) * 128, 0:nE])
                        erw = wp.tile([128, heads], BF16, tag="erw")
                        nc.sync.dma_start(out=erw[:],
                                          in_=er_tab[w * 128:(w + 1) * 128, :])
                        # er broadcast to edges: pscore[e, (c,h)] via one-hot mt
                        pscore = p1.tile([128, maxch * heads], F32, tag="pscore")
                        for c in range(nch):
                            nc.tensor.matmul(out=pscore[:, c * heads:(c + 1) * heads],
                                             lhsT=mt[:, c * 128:(c + 1) * 128],
                                             rhs=erw[:], start=(c == 0),
                                             stop=(c == nch - 1))
                        # sco = exp(lrelu(el + er))
                        xg_f32 = xg[:].bitcast(F32)
                        rw2 = row_w // 2
                        el_ap = bass.AP(xg_f32.tensor, xg_f32.offset + feats // 2,
                                        [xg_f32.ap[0], [rw2, nch], [1, heads]])
                        scof = wp.tile([128, maxch * heads], F32, tag="scof")
                        nc.vector.tensor_tensor(out=scof[:, :nch * heads],
                                                in0=pscore[:, :nch * heads],
                                                in1=el_ap, op=OP.add)
                        nc.vector.scalar_tensor_tensor(
                            out=scof[:, :nch * heads], in0=scof[:, :nch * heads],
                            scalar=NEG_SLOPE, in1=scof[:, :nch * heads],
                            op0=OP.mult, op1=OP.max)
                        sco = wp.tile([128, maxch * heads], BF16, tag="sco")
                        nc.scalar.activation(out=sco[:, :nch * heads],
                                             in_=scof[:, :nch * heads], func=AF.Exp)
                        if _ep_lvl < 2:
                            continue
                        # scaled rhs: [alpha*h | sco]
                        scaled = wp.tile([128, maxch * aggw], BF16, tag="scaled")
                        sc3 = scaled[:].rearrange("p (c r) -> p c r", r=aggw)
                        if heads > 1:
                            out_ap = bass.AP(sc3.tensor, sc3.offset,
                                             [sc3.ap[0], [aggw, nch], [Dh, heads], [1, Dh]])
                            in0_ap = bass.AP(xg3.tensor, xg3.offset,
                                             [xg3.ap[0], [row_w, nch], [Dh, heads], [1, Dh]])
                            in1_ap = bass.AP(sco[:].tensor, sco[:].offset,
                                             [sco[:].ap[0], [heads, nch], [1, heads], [0, Dh]])
                        else:
                            out_ap = bass.AP(sc3.tensor, sc3.offset,
                                             [sc3.ap[0], [aggw, nch], [1, feats]])
                            in0_ap = bass.AP(xg3.tensor, xg3.offset,
                                             [xg3.ap[0], [row_w, nch], [1, feats]])
                            in1_ap = bass.AP(sco[:].tensor, sco[:].offset,
                                             [sco[:].ap[0], [1, nch], [0, feats]])
                        nc.vector.tensor_tensor(out=out_ap, in0=in0_ap, in1=in1_ap,
                                                op=OP.mult)
                        nc.vector.tensor_copy(
                            out=bass.AP(sc3.tensor, sc3.offset + feats,
                                        [sc3.ap[0], [aggw, nch], [1, heads]]),
                            in_=sco[:, :nch * heads].rearrange(
                                "p (c h) -> p c h", h=heads))
                        if _ep_lvl < 3:
                            continue
                        # aggregate
                        pagg = p1.tile([128, aggw], F32, tag="pagg")
                        for c in range(nch):
                            nc.tensor.matmul(out=pagg[:],
                                             lhsT=mprc[:, c * 128:(c + 1) * 128],
                                             rhs=scaled[:, c * aggw:(c + 1) * aggw],
                                             start=(c == 0), stop=(c == nch - 1))
                        if _ep_lvl < 4:
                            continue
                        fin(w, pagg, wp, p2)
                        if ag_fn is not None and (w + 1) % AGCH == 0:
                            ag_fn((w + 1) // AGCH - 1)

            # ---------------- finalizers ----------------
            def make_fin12(wc, br, t_dst, er_dst, l3):
                Wc = W3 if l3 else W12
                def fin(w, pagg, wp, p2):
                    esr = wp.tile([128, H], F32, tag="esr")
                    nc.vector.tensor_scalar_max(out=esr[:], in0=pagg[:, F:F + H],
                                                scalar1=1e-30)
                    nc.vector.reciprocal(out=esr[:], in_=esr[:])
                    zb = wp.tile([128, F], F32, tag="zb")
                    esr_b = bass.AP(esr[:].tensor, esr[:].offset,
                                    [esr[:].ap[0], [1, H], [0, Dh]])
                    zb_ap = bass.AP(zb[:].tensor, zb[:].offset,
                                    [zb[:].ap[0], [Dh, H], [1, Dh]])
                    pagg_ap = bass.AP(pagg[:].tensor, pagg[:].offset,
                                      [pagg[:].ap[0], [Dh, H], [1, Dh]])
                    nc.vector.tensor_tensor(out=zb_ap, in0=pagg_ap, in1=esr_b,
                                            op=OP.mult)
                    # elu
                    e0 = wp.tile([128, F], F32, tag="e0")
                    nc.vector.tensor_scalar_min(out=e0[:], in0=zb[:], scalar1=0.0)
                    nc.scalar.activation(out=e0[:], in_=e0[:], func=AF.Exp)
                    nc.vector.tensor_scalar_add(out=e0[:], in0=e0[:], scalar1=-1.0)
                    xn = wp.tile([128, F], F32, tag="xn")
                    nc.vector.scalar_tensor_tensor(out=xn[:], in0=zb[:], scalar=0.0,
                                                   in1=e0[:], op0=OP.max, op1=OP.add)
                    if _dbg and w == 0:
                        dbg_dump(pagg[:, 0:AGG12], AGG12, wp)
                        dbg_dump(xn[:, 0:F], F, wp)
                    # next-layer table rows
                    xnT = wp.tile([128, F], BF16, tag="xnT")
                    for k in range(2):
                        pT = p2.tile([128, 128], F32, tag="pT")
                        nc.tensor.transpose(out=pT[:], in_=xn[:, k * 128:(k + 1) * 128],
                                            identity=ident[:])
                        nc.scalar.copy(out=xnT[:, k * 128:(k + 1) * 128], in_=pT[:])
                    pb = build_rows(xnT, wc, br, Wc, p2)
                    trow = wp.tile([128, fin.row_w], BF16, tag="trow")
                    erow = wp.tile([128, fin.heads], BF16, tag="erow")
                    pack_rows(pb, fin.feats, fin.heads, fin.row_w, trow, erow,
                              t_dst, er_dst, w)
                fin.needs_trow = True
                fin.row_w = ROW3 if l3 else ROW1
                fin.feats = C if l3 else F
                fin.heads = 1 if l3 else H
                return fin

            def fin3(w, pagg, wp, p2):
                esr = wp.tile([128, 1], F32, tag="esr3")
                nc.vector.tensor_scalar_max(out=esr[:], in0=pagg[:, C:C + 1],
                                            scalar1=1e-30)
                nc.vector.reciprocal(out=esr[:], in_=esr[:])
                z = wp.tile([128, C], F32, tag="z3")
                nc.vector.tensor_tensor(out=z[:], in0=pagg[:, 0:C],
                                        in1=esr[:].to_broadcast([128, C]), op=OP.mult)
                negmax = wp.tile([128, 1], F32, tag="nm")
                nc.vector.tensor_reduce(out=negmax[:], in_=z[:], axis=AX.X,
                                        op=OP.max, negate=True)
                ex = wp.tile([128, C], F32, tag="lex")
                sume = wp.tile([128, 1], F32, tag="se")
                nc.scalar.activation(out=ex[:], in_=z[:], func=AF.Exp,
                                     bias=negmax[:], accum_out=sume[:])
                lns = wp.tile([128, 1], F32, tag="ln")
                nc.scalar.activation(out=lns[:], in_=sume[:], func=AF.Ln)
                adj = wp.tile([128, 1], F32, tag="adj")
                nc.vector.tensor_tensor(out=adj[:], in0=negmax[:], in1=lns[:],
                                        op=OP.subtract)
                res = wp.tile([128, C], F32, tag="res")
                nc.vector.tensor_scalar_add(out=res[:], in0=z[:], scalar1=adj[:])
                nc.sync.dma_start(out=OUT[w * 128:(w + 1) * 128, :], in_=res[:])
            fin3.needs_trow = False

            if _stage >= 2:
                edge_phase(t1_full, ROW1, F, H, er1_tab,
                           make_fin12(w2c, b2r, t2_own, er2_tab, l3=False),
                           ag_fn=ag2)

            if _stage >= 3:
                edge_phase(t2_full, ROW1, F, H, er2_tab,
                           make_fin12(w3c, b3r, t3_own, er3_tab, l3=True),
                           ag_fn=ag3)

            if _stage >= 4:
                edge_phase(t3_full, ROW3, C, 1, er3_tab, fin3)

    lower_extended_insts(nc)
    return io


def prepare_inputs(inputs, meta, F, H, Dh, C, core):
    """Per-core in_map from full inputs + meta."""
    shard, sp = meta.shard, meta.shard_pad
    x = np.asarray(inputs['x'], np.float32)
    xo = np.zeros((sp, F), np.float32)
    xo[:shard] = x[core * shard:(core + 1) * shard]

    w1cat, b1row = fold_weights(inputs['W1'], inputs['al1'], inputs['ar1'],
                                inputs['b1'], H, Dh)
    w2cat, b2row = fold_weights(inputs['W2'], inputs['al2'], inputs['ar2'],
                                inputs['b2'], H, Dh)
    w3cat, b3row = fold_weights(inputs['W3'], inputs['al3'], inputs['ar3'],
                                inputs['b3'], 1, C)

    m = {
        'xT_own': np.ascontiguousarray(
            xo.T.reshape(2, 128, sp).transpose(1, 0, 2).reshape(128, 2 * sp)
        ).astype(BF16_NP),
        'w1cat': chunk_rows(w1cat).astype(BF16_NP),
        'w2cat': chunk_rows(w2cat).astype(BF16_NP),
        'w3cat': chunk_rows(w3cat).astype(BF16_NP),
        'b1row': b1row.reshape(1, -1).astype(BF16_NP),
        'b2row': b2row.reshape(1, -1).astype(BF16_NP),
        'b3row': b3row.reshape(1, -1).astype(BF16_NP),
        'idx16': meta.idx16[core],
        'mt': meta.mt[core],
        'mprc': meta.mprc[core],
        'ones1': np.ones((1, 128), BF16_NP),
        'ident': np.eye(128, dtype=np.float32),
    }
    return m


_CACHE = {}


def kernel(**inputs):
    import concourse.bass as bass
    from concourse.bass_utils import run_bass_kernel_spmd

    N, F, H, Dh, C, NCORES = 50000, 256, 4, 64, 40, 8
    ei = np.asarray(inputs["edge_index"])
    src = ei[0].astype(np.int64)
    dst = ei[1].astype(np.int64)

    key = "k"
    if key not in _CACHE:
        meta = build_meta(src.copy(), dst, N, NCORES)
        nc = bass.Bass("TRN2", target_bir_lowering=False, debug=False,
                       num_devices=NCORES)
        build_kernel(nc, meta, F, H, Dh, C)
        legalize_waits(nc)
        _CACHE[key] = (meta, nc)
    meta, nc = _CACHE[key]

    in_maps = [prepare_inputs(inputs, meta, F, H, Dh, C, c) for c in range(NCORES)]
    trace = os.environ.get("GAT_TRACE") == "1"
    kw = {}
    if trace:
        kw = dict(trace=True, tmpdir=os.environ.get("GAT_TRACE_DIR",
                                                    "/tmp/gat_trace"))
    res = run_bass_kernel_spmd(nc, in_maps, list(range(NCORES)), **kw)
    if trace and res.exec_time_ns is not None:
        print(f"HW exec time: {res.exec_time_ns} ns")
    sh = meta.shard
    out = np.concatenate([res.results[c]["out"][:sh] for c in range(NCORES)], 0)
    return out.astype(np.float32)


# revision 29
# speedup vs baseline: 1.2235x; 1.2235x over previous
"""3-layer GAT forward for nn_GAT_21045339750566 on 8 TRN2 NeuronCores.

Redesign vs baseline: gather bf16 *pre-transformed* per-head features
(table rows [h@W+b | el]) so edge aggregation is ONE 260-wide bf16 matmul
per 128-edge chunk; one-hot edge/dst matrices (pure graph structure) are
host-precomputed and DMA'd; biases and attention vectors are folded on
host; next-layer table build is fused into each window's finalize.

Hardcoded problem shape: N=50000 nodes, E=800000 edges, F=256, H=4 heads,
D=64, C=40 classes, 8 cores.
"""
import os
import sys
import numpy as np

sys.path.insert(0, '/opt/trn_rl_repo')

from concourse import mybir

MAX_WAITS = 1


def legalize_waits(nc, max_waits=MAX_WAITS):
    """Walrus on this stack rejects instructions carrying more than MAX_WAITS
    sem waits. Hoist excess waits onto InstNoOp instructions inserted just
    before the offending instruction (same engine, program order preserved)."""
    n_fixed = 0
    for fn in nc.m.functions:
        for blk in fn.blocks:
            il = blk.instructions
            i = 0
            while i < len(il):
                inst = il[i]
                si = inst.sync_info
                if si is not None and len(si.on_wait) > max_waits:
                    waits = list(si.on_wait)
                    keep = waits[-max_waits:]
                    extra = waits[:-max_waits]
                    inst.sync_info = mybir.SyncInfo(
                        on_wait=keep, on_update=list(si.on_update)
                    )
                    nops = []
                    for j in range(0, len(extra), max_waits):
                        nop = mybir.InstNoOp(
                            name=nc.get_next_instruction_name(),
                            engine=inst.engine,
                            bass_nofuse=True,
                            sync_info=mybir.SyncInfo(
                                on_wait=extra[j : j + max_waits], on_update=[]
                            ),
                        )
                        try:
                            nc.register_instruction(nop)
                        except Exception:
                            pass
                        nops.append(nop)
                    for k, nop in enumerate(nops):
                        il.insert(i + k, nop)
                    i += len(nops)
                    n_fixed += 1
                i += 1
    return n_fixed


import concourse.bass as bass
import concourse.tile as tile
from concourse import library_config
from concourse.library_overlay import lower_extended_insts
import ml_dtypes

BF16_NP = ml_dtypes.bfloat16
F32 = mybir.dt.float32
BF16 = mybir.dt.bfloat16
I16 = mybir.dt.int16
AF = mybir.ActivationFunctionType
OP = mybir.AluOpType
AX = mybir.AxisListType

MAXG = 2048
WIN = 128
NEG_SLOPE = 0.2
SPLIT = 32768
AGCH = 7           # windows per AllGather chunk (nwin = 49 = 7*7)
CHR = AGCH * WIN   # own rows per chunk (896)


class Meta:
    pass


def build_meta(src, dst, N, n_cores):
    """Per-core edge metadata. Per-core edge order: windows ascending; within
    a window group A (src_pad < SPLIT) then group B, each padded to a
    multiple of 128 (uniform max over cores). Pad gather idxs are -1 (the Q7
    truncates trailing negatives per core); pad slots have all-zero one-hot
    columns so they contribute nothing."""
    shard = N // n_cores
    nwin = (shard + WIN - 1) // WIN
    m = Meta()
    shard_pad = nwin * WIN
    m.N, m.n_cores, m.shard, m.nwin = N, n_cores, shard, nwin
    m.shard_pad = shard_pad
    m.N_pad = n_cores * shard_pad
    # Table rows are chunk-major so each AllGather chunk writes a contiguous
    # block: node (core g, local n) -> row (n//CHR)*(n_cores*CHR) + g*CHR + n%CHR
    g, n = src // shard, src % shard
    src = (n // CHR) * (n_cores * CHR) + g * CHR + (n % CHR)

    pcw = []   # [core][win] -> (srcA, srcB-SPLIT, dlocA, dlocB)
    for c in range(n_cores):
        sel = (dst // shard) == c
        s_c, d_c = src[sel], dst[sel]
        dloc = (d_c - c * shard).astype(np.int64)
        order = np.argsort(dloc, kind='stable')
        s_c, dloc = s_c[order], dloc[order]
        wins = []
        for w in range(nwin):
            lo, hi = np.searchsorted(dloc, [w * WIN, (w + 1) * WIN])
            sw, dw = s_c[lo:hi], dloc[lo:hi] - w * WIN
            a = sw < SPLIT
            wins.append((sw[a], sw[~a] - SPLIT, dw[a], dw[~a]))
        pcw.append(wins)

    up = lambda n: max(-(-n // 128) * 128, 0)
    nA = [max(128, max(up(len(pcw[c][w][0])) for c in range(n_cores)))
          for w in range(nwin)]
    nB = [max(up(len(pcw[c][w][1])) for c in range(n_cores)) for w in range(nwin)]

    m.win_desc = []
    icol = 0
    for w in range(nwin):
        m.win_desc.append(dict(nA=nA[w], nB=nB[w], offA=icol,
                               offB=icol + nA[w] // 16, nE=nA[w] + nB[w],
                               nch=(nA[w] + nB[w]) // 128))
        icol += (nA[w] + nB[w]) // 16
    m.tot_icols = icol
    m.maxE = max(d['nE'] for d in m.win_desc)
    m.maxch = m.maxE // 128

    def wrap16(idx):
        return np.tile(idx.reshape(-1, 16).T, (8, 1)).astype(np.int16)

    # Pad gather idxs with 0 (a valid row: every slot gets written, padded
    # slots are killed by their all-zero one-hot columns). Trailing -1
    # padding (Q7-truncated, saves desc-gen) crashes this HW stack.
    m.idx16, m.mt, m.mprc = [], [], []
    for c in range(n_cores):
        i16 = np.full((128, m.tot_icols), -1, np.int16)
        mt = np.zeros((nwin * 128, m.maxE), BF16_NP)
        mprc = np.zeros((nwin * 128, m.maxE), BF16_NP)
        for w in range(nwin):
            pad = 0
            sA, sB, dA, dB = pcw[c][w]
            d = m.win_desc[w]
            a = np.full(d['nA'], pad, np.int64); a[:len(sA)] = sA
            b = np.full(d['nB'], pad, np.int64); b[:len(sB)] = sB
            i16[:, d['offA']:d['offA'] + d['nA'] // 16] = wrap16(a)
            if d['nB']:
                i16[:, d['offB']:d['offB'] + d['nB'] // 16] = wrap16(b)
            dl = np.full(d['nE'], -1, np.int64)
            dl[:len(dA)] = dA
            dl[d['nA']:d['nA'] + len(dB)] = dB
            slots = np.nonzero(dl >= 0)[0]
            dv = dl[slots]
            # mt[j, s] = (dstloc[s] == j)
            mt[w * 128 + dv, slots] = 1
            # mprc[p, c*128 + j] = (slot c*128+p has dstloc == j)
            mprc[w * 128 + (slots % 128), (slots // 128) * 128 + dv] = 1
        m.idx16.append(i16)
        m.mt.append(mt)
        m.mprc.append(mprc)
    return m


def blockdiag(al, heads, dim):
    out = np.zeros((heads * dim, heads), np.float64)
    for h in range(heads):
        out[h * dim:(h + 1) * dim, h] = al[h]
    return out


def chunk_rows(a):
    """[K*128, W] -> [128, K*W] (row-chunk k at cols k*W)."""
    K = a.shape[0] // 128
    W = a.shape[1]
    out = np.zeros((128, K * W), a.dtype)
    for k in range(K):
        out[:, k * W:(k + 1) * W] = a[k * 128:(k + 1) * 128]
    return out


def fold_weights(W, al, ar, b, heads, dim):
    """-> (Wcat [Fin, Fout+2H] f64, brow [Fout+2H] f64)."""
    W = np.asarray(W, np.float64)
    b = np.asarray(b, np.float64).reshape(-1)
    bdl = blockdiag(np.asarray(al, np.float64), heads, dim)
    bdr = blockdiag(np.asarray(ar, np.float64), heads, dim)
    Vl, Vr = W @ bdl, W @ bdr
    bl = b @ bdl
    Wcat = np.concatenate([W, Vl, Vr], 1)
    brow = np.concatenate([b, bl, -bl])
    return Wcat, brow


def build_kernel(nc, meta, F, H, Dh, C):
    nwin, shard_pad, N_pad = meta.nwin, meta.shard_pad, meta.N_pad
    maxE, maxch = meta.maxE, meta.maxch
    ROW1 = 384   # bf16 slots per L1/L2 table row: [h(256) | el f32 (4->8 slots) | pad]
    ROW3 = 128   # L3: [h(40) | el f32 (1->2 slots) | pad]
    W12 = F + 2 * H       # 264 table-build matmul width
    W3 = C + 2            # 42
    AGG12 = F + H         # 260 agg rhs width
    AGG3 = C + 1          # 41

    io = {}
    def inp(name, shape, dtype=F32):
        io[name] = nc.dram_tensor(name, shape, dtype, kind="ExternalInput")
        return io[name]

    XT = inp("xT_own", [128, 2 * shard_pad], BF16)
    W1C = inp("w1cat", [128, 2 * W12], BF16)
    W2C = inp("w2cat", [128, 2 * W12], BF16)
    W3C = inp("w3cat", [128, 2 * W3], BF16)
    B1R = inp("b1row", [1, W12], BF16)
    B2R = inp("b2row", [1, W12], BF16)
    B3R = inp("b3row", [1, W3], BF16)
    IDX = inp("idx16", [128, meta.tot_icols], I16)
    MT = inp("mt", [nwin * 128, maxE], BF16)
    MPRC = inp("mprc", [nwin * 128, maxE], BF16)
    ONES1 = inp("ones1", [1, 128], BF16)
    IDENT = inp("ident", [128, 128])
    OUT = nc.dram_tensor("out", [shard_pad, C], F32, kind="ExternalOutput")

    nck = nwin // AGCH
    t1_own = [nc.dram_tensor(f"t1_own{k}", [CHR, ROW1], BF16) for k in range(nck)]
    t1_full = nc.dram_tensor("t1_full", [N_pad, ROW1], BF16, addr_space="Shared")
    t2_own = [nc.dram_tensor(f"t2_own{k}", [CHR, ROW1], BF16) for k in range(nck)]
    t2_full = nc.dram_tensor("t2_full", [N_pad, ROW1], BF16, addr_space="Shared")
    t3_own = [nc.dram_tensor(f"t3_own{k}", [CHR, ROW3], BF16) for k in range(nck)]
    t3_full = nc.dram_tensor("t3_full", [N_pad, ROW3], BF16, addr_space="Shared")
    er1_tab = nc.dram_tensor("er1_tab", [shard_pad, H], BF16)
    er2_tab = nc.dram_tensor("er2_tab", [shard_pad, H], BF16)
    er3_tab = nc.dram_tensor("er3_tab", [shard_pad, 1], BF16)

    _dbg = os.environ.get("GAT_DEBUG") == "1"
    if _dbg:
        DBG = nc.dram_tensor("dbg", [128, 4096], F32, kind="ExternalOutput")
        io['_dbg'] = True
        _dbg_state = {'n': 0}

    reg_cache = {}
    def reg(v):
        if v not in reg_cache:
            reg_cache[v] = nc.gpsimd.to_reg(v)
        return reg_cache[v]

    with tile.TileContext(nc) as tc:
        with tc.tile_pool(name="cst", bufs=1) as cst:
            nc.gpsimd.load_library(library_config.mlp)

            def load_const(name, shape, dtype=F32):
                tl = cst.tile(shape, dtype, tag=name)
                nc.sync.dma_start(out=tl[:], in_=io[name][:])
                return tl

            ident = load_const("ident", [128, 128])
            ones1 = load_const("ones1", [1, 128], BF16)
            w1c = load_const("w1cat", [128, 2 * W12], BF16)
            w2c = load_const("w2cat", [128, 2 * W12], BF16)
            w3c = load_const("w3cat", [128, 2 * W3], BF16)
            b1r = load_const("b1row", [1, W12], BF16)
            b2r = load_const("b2row", [1, W12], BF16)
            b3r = load_const("b3row", [1, W3], BF16)
            idx_sb = load_const("idx16", [128, meta.tot_icols], I16)

            def dbg_dump(ap, cols, pool):
                """Copy [128, cols] f32-castable AP into DBG columns."""
                if not _dbg:
                    return
                n = _dbg_state['n']
                if n + cols > 4096:
                    return
                t = pool.tile([128, cols], F32, tag=f"dbg{n}")
                nc.vector.tensor_copy(out=t[:], in_=ap)
                nc.sync.dma_start(out=DBG[:, n:n + cols], in_=t[:])
                _dbg_state['n'] = n + cols

            # table-build: psum[128, W] = xnT.T @ Wcat + 1 x brow
            def build_rows(xnT, wc, br, Wc, ppool):
                pb = ppool.tile([128, 512], F32, tag="pbuild")
                for k in range(2):
                    nc.tensor.matmul(out=pb[:, :Wc], lhsT=xnT[:, k * 128:(k + 1) * 128],
                                     rhs=wc[:, k * Wc:(k + 1) * Wc],
                                     start=(k == 0), stop=False)
                nc.tensor.matmul(out=pb[:, :Wc], lhsT=ones1[:], rhs=br[:],
                                 start=False, stop=True)
                return pb

            # pack psum rows -> bf16 table row tile + er tile, DMA out
            def pack_rows(pb, feats, heads, row_w, trow, erow, t_dst, er_dst, w):
                nc.vector.tensor_copy(out=trow[:, 0:feats], in_=pb[:, 0:feats])
                trow_f32 = trow[:].bitcast(F32)
                elc = feats // 2
                nc.vector.tensor_copy(out=trow_f32[:, elc:elc + heads],
                                      in_=pb[:, feats:feats + heads])
                nc.vector.memset(trow[:, feats + 2 * heads:row_w], 0.0)
                nc.vector.tensor_copy(out=erow[:], in_=pb[:, feats + heads:feats + 2 * heads])
                r0 = (w % AGCH) * 128
                nc.sync.dma_start(out=t_dst[w // AGCH][r0:r0 + 128, :], in_=trow[:])
                nc.sync.dma_start(out=er_dst[w * 128:(w + 1) * 128, :], in_=erow[:])

            _stage = int(os.environ.get("GAT_STAGE", "4"))

            def make_ag(t_own, t_full_t):
                def ag(k):
                    f0 = k * meta.n_cores * CHR
                    nc.gpsimd.collective_compute(
                        "AllGather", OP.bypass,
                        replica_groups=[list(range(meta.n_cores))],
                        ins=[t_own[k][:]],
                        outs=[t_full_t[f0:f0 + meta.n_cores * CHR, :]])
                return ag

            ag1 = make_ag(t1_own, t1_full) if _stage >= 1 else None
            ag2 = make_ag(t2_own, t2_full) if _stage >= 3 else None
            ag3 = make_ag(t3_own, t3_full) if _stage >= 4 else None

            # ---------------- Phase T1: own-shard table build ----------------
            with tc.tile_pool(name="bp", bufs=3) as bp, \
                 tc.tile_pool(name="bpp", bufs=2, space="PSUM") as bpp:
                for t in range(nwin):
                    xtt = bp.tile([128, 256], BF16, tag="xtt")
                    xt3 = XT[:].rearrange("p (a n) -> p a n", a=2)
                    nc.sync.dma_start(out=xtt[:].rearrange("p (a n) -> p a n", a=2),
                                      in_=xt3[:, :, t * 128:(t + 1) * 128])
                    pb = build_rows(xtt, w1c, b1r, W12, bpp)
                    trow = bp.tile([128, ROW1], BF16, tag="trow")
                    erow = bp.tile([128, H], BF16, tag="erow")
                    pack_rows(pb, F, H, ROW1, trow, erow, t1_own, er1_tab, t)
                    if ag1 is not None and (t + 1) % AGCH == 0:
                        ag1((t + 1) // AGCH - 1)

            # ---------------- edge phase ----------------
            _ep = os.environ.get("GAT_EP", "full")  # gather|score|scaled|agg|full
            _ep_lvl = ["gather", "score", "scaled", "agg", "full"].index(_ep)

            def edge_phase(table, row_w, feats, heads, er_tab, fin, ag_fn=None):
                aggw = feats + heads
                with tc.tile_pool(name="gp", bufs=4) as gp, \
                     tc.tile_pool(name="wp", bufs=2) as wp, \
                     tc.tile_pool(name="mtp", bufs=3) as mtp, \
                     tc.tile_pool(name="p1", bufs=2, space="PSUM") as p1, \
                     tc.tile_pool(name="p2", bufs=2, space="PSUM") as p2:
                    for w in range(nwin):
                        d = meta.win_desc[w]
                        nch, nE = d['nch'], d['nE']
                        xg = gp.tile([128, maxch * row_w], BF16, tag="xg")
                        xg3 = xg[:].rearrange("p (c r) -> p c r", r=row_w)
                        for (cnt, coff, ch0, base) in (
                                (d['nA'], d['offA'], 0, 0),
                                (d['nB'], d['offB'], d['nA'] // 128, SPLIT)):
                            done = 0
                            while done < cnt:
                                step = min(MAXG, cnt - done)
                                src_ap = (table[0:SPLIT, :] if base == 0
                                          else table[SPLIT:N_pad, :])
                                nc.gpsimd.dma_gather(
                                    out_ap=xg3[:, ch0 + done // 128:
                                               ch0 + (done + step) // 128, :],
                                    in_ap=src_ap,
                                    idxs_ap=idx_sb[:, coff + done // 16:
                                                   coff + (done + step) // 16],
                                    num_idxs=step, num_idxs_reg=reg(step),
                                    elem_size=row_w, single_packet=False)
                                done += step
                        if _ep_lvl < 1:
                            continue
                        # mt/mprc on the ACT HWDGE ring so a stalled transfer
                        # can't head-block the Sync ring (er/table-out DMAs)
                        mt = mtp.tile([128, maxE], BF16, tag="mt")
                        nc.scalar.dma_start(out=mt[:, :nE],
                                            in_=MT[w * 128:(w + 1) * 128, 0:nE])
                        mprc = mtp.tile([128, maxE], BF16, tag="mprc")
                        nc.scalar.dma_start(out=mprc[:, :nE],
                                            in_=MPRC[w * 128:(w + 1) * 128, 0:nE])
                        erw = wp.tile([128, heads], BF16, tag="erw")
                        nc.sync.dma_start(out=erw[:],
                                          in_=er_tab[w * 128:(w + 1) * 128, :])
                        # er broadcast to edges: pscore[e, (c,h)] via one-hot mt
                        pscore = p1.tile([128, maxch * heads], F32, tag="pscore")
                        for c in range(nch):
                            nc.tensor.matmul(out=pscore[:, c * heads:(c + 1) * heads],
                                             lhsT=mt[:, c * 128:(c + 1) * 128],
                                             rhs=erw[:], start=(c == 0),
                                             stop=(c == nch - 1))
                        # sco = exp(lrelu(el + er))
                        xg_f32 = xg[:].bitcast(F32)
                        rw2 = row_w // 2
                        el_ap = bass.AP(xg_f32.tensor, xg_f32.offset + feats // 2,
                                        [xg_f32.ap[0], [rw2, nch], [1, heads]])
                        scof = wp.tile([128, maxch * heads], F32, tag="scof")
                        nc.vector.tensor_tensor(out=scof[:, :nch * heads],
                                                in0=pscore[:, :nch * heads],
                                                in1=el_ap, op=OP.add)
                        nc.vector.scalar_tensor_tensor(
                            out=scof[:, :nch * heads], in0=scof[:, :nch * heads],
                            scalar=NEG_SLOPE, in1=scof[:, :nch * heads],
                            op0=OP.mult, op1=OP.max)
                        sco = wp.tile([128, maxch * heads], BF16, tag="sco")
                        nc.scalar.activation(out=sco[:, :nch * heads],
                                             in_=scof[:, :nch * heads], func=AF.Exp)
                        if _ep_lvl < 2:
                            continue
                        # scaled rhs: [alpha*h | sco]
                        scaled = wp.tile([128, maxch * aggw], BF16, tag="scaled")
                        sc3 = scaled[:].rearrange("p (c r) -> p c r", r=aggw)
                        if heads > 1:
                            out_ap = bass.AP(sc3.tensor, sc3.offset,
                                             [sc3.ap[0], [aggw, nch], [Dh, heads], [1, Dh]])
                            in0_ap = bass.AP(xg3.tensor, xg3.offset,
                                             [xg3.ap[0], [row_w, nch], [Dh, heads], [1, Dh]])
                            in1_ap = bass.AP(sco[:].tensor, sco[:].offset,
                                             [sco[:].ap[0], [heads, nch], [1, heads], [0, Dh]])
                        else:
                            out_ap = bass.AP(sc3.tensor, sc3.offset,
                                             [sc3.ap[0], [aggw, nch], [1, feats]])
                            in0_ap = bass.AP(xg3.tensor, xg3.offset,
                                             [xg3.ap[0], [row_w, nch], [1, feats]])
                            in1_ap = bass.AP(sco[:].tensor, sco[:].offset,
                                             [sco[:].ap[0], [1, nch], [0, feats]])
                        nc.vector.tensor_tensor(out=out_ap, in0=in0_ap, in1=in1_ap,
                                                op=OP.mult)
                        nc.vector.tensor_copy(
                            out=bass.AP(sc3.tensor, sc3.offset + feats,
                                        [sc3.ap[0], [aggw, nch], [1, heads]]),
                            in_=sco[:, :nch * heads].rearrange(
                                "p (c h) -> p c h", h=heads))
                        if _ep_lvl < 3:
                            continue
                        # aggregate
                        pagg = p1.tile([128, aggw], F32, tag="pagg")
                        for c in range(nch):
                            nc.tensor.matmul(out=pagg[:],
                                             lhsT=mprc[:, c * 128:(c + 1) * 128],
                                             rhs=scaled[:, c * aggw:(c + 1) * aggw],
                                             start=(c == 0), stop=(c == nch - 1))
                        if _ep_lvl < 4:
                            continue
                        fin(w, pagg, wp, p2)
                        if ag_fn is not None and (w + 1) % AGCH == 0:
                            ag_fn((w + 1) // AGCH - 1)

            # ---------------- finalizers ----------------
            def make_fin12(wc, br, t_dst, er_dst, l3):
                Wc = W3 if l3 else W12
                def fin(w, pagg, wp, p2):
                    esr = wp.tile([128, H], F32, tag="esr")
                    nc.vector.tensor_scalar_max(out=esr[:], in0=pagg[:, F:F + H],
                                                scalar1=1e-30)
                    nc.vector.reciprocal(out=esr[:], in_=esr[:])
                    zb = wp.tile([128, F], F32, tag="zb")
                    esr_b = bass.AP(esr[:].tensor, esr[:].offset,
                                    [esr[:].ap[0], [1, H], [0, Dh]])
                    zb_ap = bass.AP(zb[:].tensor, zb[:].offset,
                                    [zb[:].ap[0], [Dh, H], [1, Dh]])
                    pagg_ap = bass.AP(pagg[:].tensor, pagg[:].offset,
                                      [pagg[:].ap[0], [Dh, H], [1, Dh]])
                    nc.vector.tensor_tensor(out=zb_ap, in0=pagg_ap, in1=esr_b,
                                            op=OP.mult)
                    # elu
                    e0 = wp.tile([128, F], F32, tag="e0")
                    nc.vector.tensor_scalar_min(out=e0[:], in0=zb[:], scalar1=0.0)
                    nc.scalar.activation(out=e0[:], in_=e0[:], func=AF.Exp)
                    nc.vector.tensor_scalar_add(out=e0[:], in0=e0[:], scalar1=-1.0)
                    xn = wp.tile([128, F], F32, tag="xn")
                    nc.vector.scalar_tensor_tensor(out=xn[:], in0=zb[:], scalar=0.0,
                                                   in1=e0[:], op0=OP.max, op1=OP.add)
                    if _dbg and w == 0:
                        dbg_dump(pagg[:, 0:AGG12], AGG12, wp)
                        dbg_dump(xn[:, 0:F], F, wp)
                    # next-layer table rows
                    xnT = wp.tile([128, F], BF16, tag="xnT")
                    for k in range(2):
                        pT = p2.tile([128, 128], F32, tag="pT")
                        nc.tensor.transpose(out=pT[:], in_=xn[:, k * 128:(k + 1) * 128],
                                            identity=ident[:])
                        nc.scalar.copy(out=xnT[:, k * 128:(k + 1) * 128], in_=pT[:])
                    pb = build_rows(xnT, wc, br, Wc, p2)
                    trow = wp.tile([128, fin.row_w], BF16, tag="trow")
                    erow = wp.tile([128, fin.heads], BF16, tag="erow")
                    pack_rows(pb, fin.feats, fin.heads, fin.row_w, trow, erow,
                              t_dst, er_dst, w)
                fin.needs_trow = True
                fin.row_w = ROW3 if l3 else ROW1
                fin.feats = C if l3 else F
                fin.heads = 1 if l3 else H
                return fin

            def fin3(w, pagg, wp, p2):
                esr = wp.tile([128, 1], F32, tag="esr3")
                nc.vector.tensor_scalar_max(out=esr[:], in0=pagg[:, C:C + 1],
                                            scalar1=1e-30)
                nc.vector.reciprocal(out=esr[:], in_=esr[:])
                z = wp.tile([128, C], F32, tag="z3")
                nc.vector.tensor_tensor(out=z[:], in0=pagg[:, 0:C],
                                        in1=esr[:].to_broadcast([128, C]), op=OP.mult)
                negmax = wp.tile([128, 1], F32, tag="nm")
                nc.vector.tensor_reduce(out=negmax[:], in_=z[:], axis=AX.X,
                                        op=OP.max, negate=True)
                ex = wp.tile([128, C], F32, tag="lex")
                sume = wp.tile([128, 1], F32, tag="se")
                nc.scalar.activation(out=ex[:], in_=z[:], func=AF.Exp,
                                     bias=negmax[:], accum_out=sume[:])
                lns = wp.tile([128, 1], F32, tag="ln")
                nc.scalar.activation(out=lns[:], in_=sume[:], func=AF.Ln)
                adj = wp.tile([128, 1], F32, tag="adj")
                nc.vector.tensor_tensor(out=adj[:], in0=negmax[:], in1=lns[:],
                                        op=OP.subtract)
                res = wp.tile([128, C], F32, tag="res")
                nc.vector.tensor_scalar_add(out=res[:], in0=z[:], scalar1=adj[:])
                nc.sync.dma_start(out=OUT[w * 128:(w + 1) * 128, :], in_=res[:])
            fin3.needs_trow = False

            if _stage >= 2:
                edge_phase(t1_full, ROW1, F, H, er1_tab,
                           make_fin12(w2c, b2r, t2_own, er2_tab, l3=False),
                           ag_fn=ag2)

            if _stage >= 3:
                edge_phase(t2_full, ROW1, F, H, er2_tab,
                           make_fin12(w3c, b3r, t3_own, er3_tab, l3=True),
                           ag_fn=ag3)

            if _stage >= 4:
                edge_phase(t3_full, ROW3, C, 1, er3_tab, fin3)

    lower_extended_insts(nc)
    return io


def prepare_inputs(inputs, meta, F, H, Dh, C, core):
    """Per-core in_map from full inputs + meta."""
    shard, sp = meta.shard, meta.shard_pad
    x = np.asarray(inputs['x'], np.float32)
    xo = np.zeros((sp, F), np.float32)
    xo[:shard] = x[core * shard:(core + 1) * shard]

    w1cat, b1row = fold_weights(inputs['W1'], inputs['al1'], inputs['ar1'],
                                inputs['b1'], H, Dh)
    w2cat, b2row = fold_weights(inputs['W2'], inputs['al2'], inputs['ar2'],
                                inputs['b2'], H, Dh)
    w3cat, b3row = fold_weights(inputs['W3'], inputs['al3'], inputs['ar3'],
                                inputs['b3'], 1, C)

    m = {
        'xT_own': np.ascontiguousarray(
            xo.T.reshape(2, 128, sp).transpose(1, 0, 2).reshape(128, 2 * sp)
        ).astype(BF16_NP),
        'w1cat': chunk_rows(w1cat).astype(BF16_NP),
        'w2cat': chunk_rows(w2cat).astype(BF16_NP),
        'w3cat': chunk_rows(w3cat).astype(BF16_NP),
        'b1row': b1row.reshape(1, -1).astype(BF16_NP),
        'b2row': b2row.reshape(1, -1).astype(BF16_NP),
        'b3row': b3row.reshape(1, -1).astype(BF16_NP),
        'idx16': meta.idx16[core],
        'mt': meta.mt[core],
        'mprc': meta.mprc[core],
        'ones1': np.ones((1, 128), BF16_NP),
        'ident': np.eye(128, dtype=np.float32),
    }
    return m


_CACHE = {}


def kernel(**inputs):
    import concourse.bass as bass
    from concourse.bass_utils import run_bass_kernel_spmd

    N, F, H, Dh, C, NCORES = 50000, 256, 4, 64, 40, 8
    ei = np.asarray(inputs["edge_index"])
    src = ei[0].astype(np.int64)
    dst = ei[1].astype(np.int64)

    key = "k"
    if key not in _CACHE:
        meta = build_meta(src.copy(), dst, N, NCORES)
        nc = bass.Bass("TRN2", target_bir_lowering=False, debug=False,
                       num_devices=NCORES)
        build_kernel(nc, meta, F, H, Dh, C)
        legalize_waits(nc)
        _CACHE[key] = (meta, nc)
    meta, nc = _CACHE[key]

    in_maps = [prepare_inputs(inputs, meta, F, H, Dh, C, c) for c in range(NCORES)]
    trace = os.environ.get("GAT_TRACE") == "1"
    kw = {}
    if trace:
        kw = dict(trace=True, tmpdir=os.environ.get("GAT_TRACE_DIR",
                                                    "/tmp/gat_trace"))
    res = run_bass_kernel_spmd(nc, in_maps, list(range(NCORES)), **kw)
    if trace and res.exec_time_ns is not None:
        print(f"HW exec time: {res.exec_time_ns} ns")
    sh = meta.shard
    out = np.concatenate([res.results[c]["out"][:sh] for c in range(NCORES)], 0)
    return out.astype(np.float32)
